# revision 17
# baseline (speedup 1.0000x reference)
"""BERT-base forward on 8 Trainium2 NeuronCores.

Strategy: pure data parallelism over the batch (B=16 -> 2 sequences per
core), weights replicated, zero collectives.  Inside each core the
activation stream alternates between token-major layout (for layernorm,
whose reduction runs along the free dimension) and feature-major layout
(for feeding the PE array), with PE-transposes bridging the two.

v9 (vs v8): phase boundaries overlap.  The attention output projection
is emitted inside the attention psum pool right behind the softmax
epilogue; layernorm finalize is split into token-quad halves and the
token->feature transposes are quad-major, so FFN1's nf=0 half starts as
soon as the first 4 token chunks are normalized+transposed while the
other half is still finishing.  Same structure at the FFN2->LN2
boundary so the next layer's V/QK projections overlap the LN2 tail.
One layer-wide PSUM pool: tag "sc" = 2x 4KiB slots (scoresT pairs,
out-proj, FFN2), tag "cx" = 4x 2KiB slots (QK proj, ctx, transposes,
FFN1).

v8: attention is software-pipelined by head pair with fine-grained
interleave: scores(p) units (2 concurrent matmuls + ONE fused exp over
both heads of the pair) are spaced by filler units [ctx(p-1), QK(p+1),
ctxT-transpose(p-2)], keeping the PE dense while the ScalarE exp
conveyor drains.  scores are computed TRANSPOSED
(scoresT[k,q] = K_slice.T @ Q) so exp output is already in ctx-matmul
lhsT layout and no probs transpose is needed; V carries a ones column
per head so the ctx psum col 64 holds the softmax row-sum and
normalization is a per-partition DVE multiply.  Layernorm: batched -
centered square-sum via ScalarE Square(bias=-mean) (table-set filler,
no ACT_TABLE_LOAD), one Sqrt per quad + DVE reciprocal.

Precision: matmul operands are bf16 (PSUM accumulation is fp32), the
residual stream / layernorm statistics / softmax stay fp32.

Layout conventions per core (P=128 partitions):
  tokens NT=1024 (2 seqs x 512), token chunk tc in [0,8)
  features H=768, feature chunk hc in [0,6); FFN I=3072, ic in [0,24)
  token-major  [128 tokens, H]  - residual stream, layernorm, ctx, V
  feature-major [128 features, NT] - matmul lhsT/rhs operands
  matmul computes out = lhsT.T @ rhs (contraction along partitions)
"""

import numpy as np
import ml_dtypes

V, H, L, NH, I, S = 30522, 768, 12, 12, 3072, 512
B_FULL, NCORES, B_LOC = 16, 8, 2
DH = H // NH                      # 64
P = 128
NT = B_LOC * S                    # 1024 tokens per core
TC = NT // P                      # 8 token chunks
HC = H // P                       # 6 feature chunks
IC = I // P                       # 24 ffn chunks
SC = S // P                       # 4 chunks per sequence
EPS = 1e-12
INV_SQRT_DH = 1.0 / 8.0

_BF16 = ml_dtypes.bfloat16


# --------------------------------------------------------------------------
# device kernel builder
# --------------------------------------------------------------------------

def build(layers=L, taps=None, with_mask=False, with_brow=False):
    import concourse.bass as bass
    import concourse.mybir as mybir
    import concourse.tile as tile
    from concourse import bacc
    from contextlib import ExitStack

    dt = mybir.dt
    AF = mybir.ActivationFunctionType
    OP = mybir.AluOpType

    nc = bacc.Bacc("TRN2", target_bir_lowering=False, debug=False,
                   num_devices=NCORES)

    # ---- DRAM inputs (per core) ----
    identm = nc.dram_tensor("identm", [P, P], dt.bfloat16, kind="ExternalInput")
    wrows = nc.dram_tensor("wrows", [NT, H], dt.bfloat16, kind="ExternalInput")
    trows = nc.dram_tensor("trows", [NT, H], dt.bfloat16, kind="ExternalInput")
    pemb = nc.dram_tensor("pemb", [S, H], dt.float32, kind="ExternalInput")
    extm = nc.dram_tensor("extm", [1, B_LOC * S], dt.float32, kind="ExternalInput")
    dWq = nc.dram_tensor("Wq", [L, H, H], dt.bfloat16, kind="ExternalInput")
    dWk = nc.dram_tensor("Wk", [L, H, H], dt.bfloat16, kind="ExternalInput")
    dWv = nc.dram_tensor("Wv", [L, H, H], dt.bfloat16, kind="ExternalInput")
    dWo = nc.dram_tensor("Wo", [L, H, H], dt.bfloat16, kind="ExternalInput")
    dW1 = nc.dram_tensor("W1", [L, H, I], dt.bfloat16, kind="ExternalInput")
    dW2 = nc.dram_tensor("W2", [L, I, H], dt.bfloat16, kind="ExternalInput")
    # per-partition biases: bq is pre-scaled by 1/sqrt(DH) host-side
    dbq = nc.dram_tensor("bq8", [L, H], dt.float32, kind="ExternalInput")
    dbk = nc.dram_tensor("bk", [L, H], dt.float32, kind="ExternalInput")
    db1 = nc.dram_tensor("b1", [L, I], dt.float32, kind="ExternalInput")
    # free-dim biases (added via K=1 rank-1 matmuls): rows [bo + bv@Wo, b2]
    dbrow = nc.dram_tensor("brow", [L, 1, 2 * H], dt.bfloat16, kind="ExternalInput")
    out = nc.dram_tensor("out", [NT, H], dt.float32, kind="ExternalOutput")

    f32, bf16 = dt.float32, dt.bfloat16

    def tap(name, tiles):
        if taps is None:
            return
        sh0 = list(tiles[0].shape)
        d = nc.dram_tensor(f"tap_{name}", [len(tiles)] + sh0,
                           tiles[0].dtype, kind="ExternalOutput")
        for i, t in enumerate(tiles):
            nc.sync.dma_start(d.ap()[i], t[:])
        taps[name] = d

    with tile.TileContext(nc) as tc_, ExitStack() as top:
        tc = tc_

        # ---- constants & persistent activation tiles ----
        pers = top.enter_context(tc.tile_pool(name="pers", bufs=1))
        ident = pers.tile([P, P], bf16, name="ident")
        nc.sync.dma_start(ident[:], identm.ap())
        ones1 = pers.tile([1, P], bf16, name="ones1")
        nc.vector.memset(ones1[:], 1.0)
        eps_t = pers.tile([P, 1], f32, name="eps_t")
        nc.vector.memset(eps_t[:], EPS)

        curA = [pers.tile([P, H], f32, name=f"curA{t}") for t in range(TC)]
        curB = [pers.tile([P, H], f32, name=f"curB{t}") for t in range(TC)]
        xtok = [pers.tile([P, H], bf16, name=f"xtok{t}") for t in range(TC)]
        xT = [pers.tile([P, NT], bf16, name=f"xT{h}") for h in range(HC)]
        # token-major V with a ones column per head: [:, hd, 0:64]=V,
        # [:, hd, 64]=1.0 (set once; V copies never touch col 64)
        vaug = [pers.tile([P, NH, DH + 1], bf16, name=f"vaug{t}")
                for t in range(TC)]
        for t in range(TC):
            nc.vector.memset(vaug[t][:], 1.0)

        small = top.enter_context(tc.tile_pool(name="small", bufs=6))

        # next-layer Wq/Wv prefetch pool: layer l+1's tiles are DMA'd during
        # layer l's FFN phase (before FFN2's W2 burst) so the attention
        # prologue never waits on weight DMA.
        pre_pool = top.enter_context(tc.tile_pool(name="prew", bufs=12))
        pre_w = {}

        def prefetch_qv(l):
            if l >= layers:
                return
            wq = [pre_pool.tile([P, H], bf16, tag="pw", name=f"pwq{l}_{h}")
                  for h in range(HC)]
            wv = [pre_pool.tile([P, H], bf16, tag="pw", name=f"pwv{l}_{h}")
                  for h in range(HC)]
            for h in range(HC):
                nc.sync.dma_start(wq[h][:], dWq.ap()[l, h * P:(h + 1) * P, :])
                nc.sync.dma_start(wv[h][:], dWv.ap()[l, h * P:(h + 1) * P, :])
            pre_w[l] = (wq, wv)

        # per-key-chunk mask columns for the exp bias (token-major [P, TC])
        mask_cols = None
        if with_mask:
            extm_sb = pers.tile([1, B_LOC * S], f32, name="extm_sb")
            nc.sync.dma_start(extm_sb[:], extm.ap())
            mask_cols = pers.tile([P, TC], f32, name="mask_cols")
            ones_bf = pers.tile([1, 1], bf16, name="ones_bf")
            nc.vector.memset(ones_bf[:], 1.0)
            extm_bf = pers.tile([1, B_LOC * S], bf16, name="extm_bf")
            nc.vector.tensor_copy(extm_bf[:], extm_sb[:])
            with ExitStack() as ms:
                mp = ms.enter_context(
                    tc.tile_pool(name="maskp", space="PSUM", bufs=1))
                for t in range(TC):
                    mps = mp.tile([P, 1], f32, tag="m", bufs=2, name="mps")
                    nc.tensor.matmul(
                        mps[:], lhsT=extm_bf[0:1, t * P:(t + 1) * P],
                        rhs=ones_bf[:], start=True, stop=True)
                    nc.scalar.copy(mask_cols[:, t:t + 1], mps[:])

        # ---------------- helpers ----------------
        # Batched layernorm with partial (token-quad) finalize.
        class LNBatch:
            def __init__(self):
                self.uneg = small.tile([P, TC], f32, tag="uneg", bufs=2,
                                       name="uneg")
                self.svar = small.tile([P, TC], f32, tag="svar", bufs=2,
                                       name="svar")
                self.rstd = small.tile([P, TC], f32, tag="rstd", bufs=2,
                                       name="rstd")
                self.items = []

            def add(self, src_ap, res_ap, dst, tcid, last=False, out_f32=None):
                j = len(self.items)
                s1 = small.tile([P, 1], f32, tag="s1")
                nc.vector.scalar_tensor_tensor(
                    out=dst[:], in0=src_ap, scalar=0.0, in1=res_ap,
                    op0=OP.add, op1=OP.add, accum_out=s1[:])
                nc.vector.tensor_scalar(
                    out=self.uneg[:, j:j + 1], in0=s1[:], scalar1=-1.0 / H,
                    scalar2=None, op0=OP.mult)
                junk = small.tile([P, H], f32, tag="junk", bufs=2)
                nc.scalar.activation(junk[:], dst[:], AF.Square,
                                     bias=self.uneg[:, j:j + 1],
                                     accum_out=self.svar[:, j:j + 1])
                self.items.append((dst, tcid, last, out_f32))

            def finish(self, a, b):
                sd = small.tile([P, TC], f32, tag="sd", bufs=2, name="sd")
                nc.scalar.activation(sd[:, a:b], self.svar[:, a:b], AF.Sqrt,
                                     bias=eps_t[:], scale=1.0 / H)
                nc.vector.reciprocal(self.rstd[:, a:b], sd[:, a:b])
                for j in range(a, b):
                    dst, tcid, last, out_f32 = self.items[j]
                    nc.vector.tensor_scalar(
                        out=dst[:], in0=dst[:], scalar1=self.uneg[:, j:j + 1],
                        scalar2=self.rstd[:, j:j + 1],
                        op0=OP.add, op1=OP.mult)
                    if last:
                        nc.sync.dma_start(out_f32, dst[:])
                    else:
                        nc.vector.tensor_copy(xtok[tcid][:], dst[:])

        def trans_quad(pool, tag, bufs, srcs, dsts, tq, keng=0):
            """One token quad (4 chunks) of token-major srcs -> columns
            [tq*512, (tq+1)*512) of all 6 feature-major dsts."""
            for h in range(HC):
                pt = pool.tile([P, 4, P], f32, tag=tag, bufs=bufs, name="pt")
                for j in range(4):
                    t = tq * 4 + j
                    nc.tensor.matmul(pt[:, j, :],
                                     lhsT=srcs[t][:, h * P:(h + 1) * P],
                                     rhs=ident[:], start=True, stop=True)
                dst = dsts[h][:, tq * 4 * P:(tq * 4 + 4) * P]
                src = pt[:].rearrange("p a b -> p (a b)")
                if (h + keng) % 2 == 0:
                    nc.vector.tensor_copy(dst, src)
                else:
                    nc.scalar.copy(dst, src)

        # ---- embedding: gather + add + LN ----
        with ExitStack() as emb_scope:
            ep = emb_scope.enter_context(tc.tile_pool(name="emb", bufs=1))
            epp = emb_scope.enter_context(
                tc.tile_pool(name="embp", space="PSUM", bufs=1))
            wg = ep.tile([P, TC, H], bf16, name="wg")
            tg = ep.tile([P, TC, H], bf16, name="tg")
            nc.sync.dma_start(wg[:], wrows.ap().rearrange("(c p) h -> p c h", p=P))
            nc.sync.dma_start(tg[:], trows.ap().rearrange("(c p) h -> p c h", p=P))
            pos = ep.tile([P, SC, H], f32, name="pos")
            nc.sync.dma_start(pos[:], pemb.ap().rearrange("(c p) h -> p c h", p=P))
            prefetch_qv(0)
            lb = LNBatch()
            for t in range(TC):
                tmp = ep.tile([P, H], f32, tag="etmp", bufs=2, name="etmp")
                nc.vector.tensor_add(tmp[:], tg[:, t], pos[:, t % SC])
                lb.add(wg[:, t], tmp[:], curA[t], t)
            lb.finish(0, TC)
            for tq in range(2):
                trans_quad(epp, "t", 2, xtok, xT, tq)
            tap("emb", curA)
            tap("embxT", xT)

        # ---- transformer layers ----
        for l in range(layers):
            with ExitStack() as ls:
                wp = ls.enter_context(tc.tile_pool(name=f"w{l}", bufs=1))
                psL = ls.enter_context(
                    tc.tile_pool(name=f"psL{l}", space="PSUM", bufs=1))
                # per-partition bias tiles for this layer
                bq_t = wp.tile([P, HC], f32, name=f"bq{l}")
                bk_t = wp.tile([P, HC], f32, name=f"bk{l}")
                b1_t = wp.tile([P, IC], f32, name=f"b1{l}")
                nc.sync.dma_start(bq_t[:], dbq.ap()[l].rearrange("(c p) -> p c", p=P))
                nc.sync.dma_start(bk_t[:], dbk.ap()[l].rearrange("(c p) -> p c", p=P))
                nc.sync.dma_start(b1_t[:], db1.ap()[l].rearrange("(c p) -> p c", p=P))
                brow_t = wp.tile([1, 2 * H], bf16, name=f"brow{l}")
                nc.sync.dma_start(brow_t[:], dbrow.ap()[l])

                cur, nxt = (curA, curB)

                def sc_tile(shape, name):
                    return psL.tile(shape, f32, tag="sc", bufs=2, name=name)

                def cx_tile(shape, name):
                    return psL.tile(shape, f32, tag="cx", bufs=4, name=name)

                with ExitStack() as attn_scope:
                    ap_ = attn_scope.enter_context(
                        tc.tile_pool(name=f"attn{l}", bufs=1))
                    wpool = attn_scope.enter_context(
                        tc.tile_pool(name=f"wqkv{l}", bufs=12))

                    QT = [ap_.tile([P, NT], bf16, name=f"QT{l}_{h}") for h in range(HC)]
                    KT = [ap_.tile([P, NT], bf16, name=f"KT{l}_{h}") for h in range(HC)]
                    ctok = [ap_.tile([P, H], bf16, name=f"ctok{l}_{t}")
                            for t in range(TC)]
                    ctxT = [ap_.tile([P, NT], bf16, name=f"cT{l}_{h}") for h in range(HC)]

                    wqch, wvch = pre_w.pop(l)
                    wkch = [wpool.tile([P, H], bf16, tag="wc",
                                       name=f"wk{l}_{h}") for h in range(HC)]
                    for h in range(HC):
                        nc.sync.dma_start(wkch[h][:], dWk.ap()[l, h * P:(h + 1) * P, :])

                    # ---- emission units for the software pipeline ----
                    def v_unit(t, spec):
                        nf, n0, nn, h0, nh = spec
                        ps = sc_tile([P, S], "psv")
                        for hi in range(HC):
                            nc.tensor.matmul(
                                ps[:, :nn],
                                lhsT=xT[hi][:, t * P:(t + 1) * P],
                                rhs=wvch[hi][:, n0:n0 + nn],
                                start=(hi == 0), stop=(hi == HC - 1))
                        nc.vector.tensor_copy(
                            vaug[t][:, h0:h0 + nh, 0:DH],
                            ps[:, :nn].rearrange("p (h d) -> p h d", h=nh))

                    def qk_unit(p, dW_ch, bt, scale, dstT, nf):
                        ps = cx_tile([P, S], "psqk")
                        for hi in range(HC):
                            nc.tensor.matmul(
                                ps[:],
                                lhsT=dW_ch[hi][:, p * P:(p + 1) * P],
                                rhs=xT[hi][:, nf * S:(nf + 1) * S],
                                start=(hi == 0), stop=(hi == HC - 1))
                        nc.vector.tensor_scalar(
                            out=dstT[p][:, nf * S:(nf + 1) * S],
                            in0=ps[:], scalar1=bt[:, p:p + 1],
                            scalar2=float(scale), op0=OP.add, op1=OP.mult)

                    QK_SPECS = ((wqch, bq_t, INV_SQRT_DH, QT),
                                (wkch, bk_t, 1.0, KT))

                    def qk_units(p, which=(0, 1), nfs=(0, 1)):
                        return [(lambda dc=dc, bt=bt, sc_=sc_, dT=dT,
                                 nf=nf: qk_unit(p, dc, bt, sc_, dT, nf))
                                for dc, bt, sc_, dT in
                                [QK_SPECS[w] for w in which]
                                for nf in nfs]

                    def ctx_unit(ex2, s, side, hd):
                        cx = cx_tile([P, SC, DH + 1], "cx")
                        for qc in range(SC):
                            for kc in range(SC):
                                nc.tensor.matmul(
                                    cx[:, qc, :],
                                    lhsT=ex2[:, kc, side, qc * P:(qc + 1) * P],
                                    rhs=vaug[s * SC + kc][:, hd, :],
                                    start=(kc == 0), stop=(kc == SC - 1))
                        cxs = small.tile([P, SC, DH + 1], f32, tag="cxs",
                                         bufs=4)
                        nc.vector.tensor_copy(cxs[:], cx[:])
                        rinv = small.tile([P, SC], f32, tag="rinv", bufs=4)
                        nc.vector.reciprocal(rinv[:], cxs[:, :, DH])
                        for qc in range(SC):
                            nc.vector.tensor_scalar(
                                out=ctok[s * SC + qc][:, hd * DH:(hd + 1) * DH],
                                in0=cxs[:, qc, 0:DH],
                                scalar1=rinv[:, qc:qc + 1],
                                scalar2=None, op0=OP.mult)

                    def trans_unit(h):
                        # ctxT[h] only needs head pair h of ctok
                        for tq in range(TC // 4):
                            pt = cx_tile([P, 4, P], "ptc")
                            for j in range(4):
                                t = tq * 4 + j
                                nc.tensor.matmul(
                                    pt[:, j, :],
                                    lhsT=ctok[t][:, h * P:(h + 1) * P],
                                    rhs=ident[:], start=True, stop=True)
                            nc.vector.tensor_copy(
                                ctxT[h][:, tq * 4 * P:(tq * 4 + 4) * P],
                                pt[:].rearrange("p a b -> p (a b)"))

                    # ---- prologue, ordered by token quad and DMA arrival:
                    # Q/V use prefetched weights; K's DMA finishes during V
                    for u in qk_units(0, which=(0,), nfs=(0,)):
                        u()
                    for t in range(SC):
                        v_unit(t, (0, 0, S, 0, 8))
                        v_unit(t, (1, S, H - S, 8, 4))
                    for u in qk_units(0, which=(0,), nfs=(1,)):
                        u()
                    for t in range(SC, TC):
                        v_unit(t, (0, 0, S, 0, 8))
                        v_unit(t, (1, S, H - S, 8, 4))
                    for u in qk_units(0, which=(1,)):
                        u()

                    ex_pend = None   # (ex2 tiles, pair) awaiting ctx
                    for p in range(HC):
                        hA, hB = 2 * p, 2 * p + 1
                        fillers = []
                        if p == 0:
                            fillers += qk_units(1)
                        else:
                            pe, pp = ex_pend
                            for s in range(B_LOC):
                                for side, hd in ((0, 2 * pp), (1, 2 * pp + 1)):
                                    fillers.append(
                                        lambda e=pe[s], s=s, sd=side,
                                        hd=hd: ctx_unit(e, s, sd, hd))
                            if p + 1 < HC:
                                fillers += qk_units(p + 1)
                            if p - 2 >= 0:
                                fillers.append(lambda h=p - 2: trans_unit(h))

                        # score units (s, kc): 2 concurrent matmuls into a
                        # 2-bank psum slot, ONE fused exp over both heads
                        ex2s = [None, None]
                        nslots = 2 * SC
                        fi = 0
                        for s in range(B_LOC):
                            ex2s[s] = ap_.tile([P, SC, 2, S], bf16,
                                               tag="ex", bufs=3, name="ex2")
                            for kc in range(SC):
                                ps2 = sc_tile([P, 2, S], "ps2")
                                for side, hd in ((0, hA), (1, hB)):
                                    po = (hd % 2) * DH
                                    nc.tensor.matmul(
                                        ps2[:, side, :],
                                        lhsT=KT[p][po:po + DH,
                                                   s * S + kc * P:
                                                   s * S + (kc + 1) * P],
                                        rhs=QT[p][po:po + DH,
                                                  s * S:(s + 1) * S],
                                        start=True, stop=True,
                                        tile_position=(po, 0))
                                bias = (mask_cols[:, s * SC + kc:
                                                  s * SC + kc + 1]
                                        if with_mask else 0.0)
                                nc.scalar.activation(
                                    ex2s[s][:, kc, :, :].rearrange(
                                        "p a b -> p (a b)"),
                                    ps2[:].rearrange("p a b -> p (a b)"),
                                    AF.Exp, bias=bias)
                                # pop interleaved fillers
                                slot = s * SC + kc
                                want = ((slot + 1) * len(fillers)) // nslots
                                while fi < want:
                                    fillers[fi]()
                                    fi += 1
                        while fi < len(fillers):
                            fillers[fi]()
                            fi += 1
                        ex_pend = (ex2s, p)

                    # epilogue: last ctx + ctxT transposes
                    pe, pp = ex_pend
                    for s in range(B_LOC):
                        for side, hd in ((0, 2 * pp), (1, 2 * pp + 1)):
                            ctx_unit(pe[s], s, side, hd)
                    trans_unit(HC - 2)
                    trans_unit(HC - 1)

                    if l == 0:
                        tap("QT", QT)
                        tap("KT", KT)
                        tap("V", vaug)
                        tap("ctok", ctok)
                        tap("ctxT", ctxT)

                    # --- output projection + LN1, quad-split finalize ---
                    woch = [wpool.tile([P, H], bf16, tag="wc",
                                       name=f"wo{l}_{h}") for h in range(HC)]
                    for h in range(HC):
                        nc.sync.dma_start(woch[h][:], dWo.ap()[l, h * P:(h + 1) * P, :])
                    lb = LNBatch()

                    def outproj_unit(t):
                        po_ = sc_tile([P, H], "po")
                        for nf, n0, nn in ((0, 0, S), (1, S, H - S)):
                            for hi in range(HC):
                                nc.tensor.matmul(
                                    po_[:, n0:n0 + nn],
                                    lhsT=ctxT[hi][:, t * P:(t + 1) * P],
                                    rhs=woch[hi][:, n0:n0 + nn],
                                    start=(hi == 0),
                                    stop=(hi == HC - 1 and not with_brow))
                            if with_brow:
                                nc.tensor.matmul(po_[:, n0:n0 + nn], lhsT=ones1[:],
                                                 rhs=brow_t[0:1, n0:n0 + nn],
                                                 start=False, stop=True)
                        lb.add(po_[:], cur[t][:], nxt[t], t)

                    for t in range(6):
                        outproj_unit(t)
                    lb.finish(0, 4)
                    outproj_unit(6)
                    outproj_unit(7)
                    lb.finish(4, 8)
                    trans_quad(psL, "cx", 4, xtok, xT, 0, keng=0)
                    if l == 0:
                        tap("ln1", nxt)

                # --- FFN ---
                with ExitStack() as ffn_scope:
                    fp_ = ffn_scope.enter_context(
                        tc.tile_pool(name=f"ffn{l}", bufs=1))
                    w1pool = ffn_scope.enter_context(
                        tc.tile_pool(name=f"w1p{l}", bufs=6))
                    w2pool = ffn_scope.enter_context(
                        tc.tile_pool(name=f"w2p{l}", bufs=4))

                    gT = [fp_.tile([P, NT], bf16, name=f"gT{l}_{i}") for i in range(IC)]
                    w1ch = [w1pool.tile([P, I], bf16, tag="w1c",
                                        name=f"w1{l}_{h}") for h in range(HC)]
                    for h in range(HC):
                        nc.sync.dma_start(w1ch[h][:], dW1.ap()[l, h * P:(h + 1) * P, :])
                    prefetch_qv(l + 1)

                    def ffn1_half(nf):
                        for i_ in range(IC):
                            ps = cx_tile([P, S], "psf")
                            for hi in range(HC):
                                nc.tensor.matmul(
                                    ps[:],
                                    lhsT=w1ch[hi][:, i_ * P:(i_ + 1) * P],
                                    rhs=xT[hi][:, nf * S:(nf + 1) * S],
                                    start=(hi == 0), stop=(hi == HC - 1))
                            nc.scalar.activation(
                                gT[i_][:, nf * S:(nf + 1) * S], ps[:], AF.Gelu,
                                bias=b1_t[:, i_:i_ + 1], scale=1.0)

                    ffn1_half(0)
                    # second token quad of LN1 output -> xT while FFN1 nf=0 runs
                    trans_quad(psL, "cx", 4, xtok, xT, 1, keng=0)
                    ffn1_half(1)

                    if l == 0:
                        tap("gT", gT)

                    # ffn2: token pairs (2-slot psum rotation), LN2 finalize
                    # and transposes per token quad
                    lb = LNBatch()
                    last = (l == layers - 1)
                    for tp in range(TC // 2):
                        w2ch = [w2pool.tile([P, H], bf16, tag="w2c",
                                            name=f"w2_{l}_{tp}_{i}") for i in range(IC)]
                        pf = [sc_tile([P, H], f"pf{t}") for t in range(2)]
                        for i_ in range(IC):
                            nc.sync.dma_start(w2ch[i_][:], dW2.ap()[l, i_ * P:(i_ + 1) * P, :])
                        for i_ in range(IC):
                            for t in range(2):
                                tt = tp * 2 + t
                                for nf, n0, nn in ((0, 0, S), (1, S, H - S)):
                                    nc.tensor.matmul(
                                        pf[t][:, n0:n0 + nn],
                                        lhsT=gT[i_][:, tt * P:(tt + 1) * P],
                                        rhs=w2ch[i_][:, n0:n0 + nn],
                                        start=(i_ == 0),
                                        stop=(i_ == IC - 1 and not with_brow))
                        for t in range(2):
                            tt = tp * 2 + t
                            if with_brow:
                                for nf, n0, nn in ((0, 0, S), (1, S, H - S)):
                                    nc.tensor.matmul(pf[t][:, n0:n0 + nn],
                                                     lhsT=ones1[:],
                                                     rhs=brow_t[0:1, H + n0:H + n0 + nn],
                                                     start=False, stop=True)
                            lb.add(pf[t][:], nxt[tt][:], cur[tt], tt, last=last,
                                   out_f32=out.ap()[tt * P:(tt + 1) * P, :])
                        if tp == 1:
                            lb.finish(0, 4)
                            if not last:
                                trans_quad(psL, "cx", 4, xtok, xT, 0, keng=1)
                    lb.finish(4, 8)
                    if not last:
                        trans_quad(psL, "cx", 4, xtok, xT, 1, keng=1)

    nc.compile()
    return nc


# --------------------------------------------------------------------------
# host side
# --------------------------------------------------------------------------

def prep_shared(inputs):
    sh = {}
    sh["identm"] = np.eye(P, dtype=_BF16)
    sh["wemb_bf"] = inputs["word_emb"].astype(_BF16)
    sh["temb_bf"] = inputs["type_emb"].astype(_BF16)
    sh["pemb"] = inputs["pos_emb"].astype(np.float32)
    for k in ("Wq", "Wk", "Wv", "Wo", "W1", "W2"):
        sh[k] = inputs[k].astype(_BF16)
    sh["bq8"] = (inputs["bq"] * INV_SQRT_DH).astype(np.float32)
    sh["bk"] = inputs["bk"].astype(np.float32)
    sh["b1"] = inputs["b1"].astype(np.float32)
    # bv is folded into the attn-out row bias: (ctx+bv)@Wo + bo
    #   = ctx@Wo + (bv@Wo + bo)
    bo_eff = np.einsum("lh,lho->lo", inputs["bv"].astype(np.float64),
                       inputs["Wo"].astype(np.float64)) + inputs["bo"]
    sh["brow"] = np.concatenate(
        [bo_eff.astype(np.float32), inputs["b2"]], axis=1)[:, None, :].astype(_BF16)
    return sh


def core_inputs(inputs, sh, c):
    ids = np.asarray(inputs["input_ids"]).astype(np.int64)
    tts = np.asarray(inputs["token_type_ids"]).astype(np.int64)
    am = np.asarray(inputs["attention_mask"]).astype(np.float32)
    b0 = c * B_LOC
    m = {k: v for k, v in sh.items() if k not in ("wemb_bf", "temb_bf")}
    m["wrows"] = np.ascontiguousarray(sh["wemb_bf"][ids[b0:b0 + B_LOC].reshape(-1)])
    m["trows"] = np.ascontiguousarray(sh["temb_bf"][tts[b0:b0 + B_LOC].reshape(-1)])
    m["extm"] = ((1.0 - am[b0:b0 + B_LOC]) * -10000.0).reshape(1, -1).astype(np.float32)
    return m


_NC_CACHE = {}


def flags_for(inputs):
    with_mask = not np.all(np.asarray(inputs["attention_mask"]) == 1.0)
    with_brow = bool(np.any(np.asarray(inputs["bo"])) or
                     np.any(np.asarray(inputs["bv"])) or
                     np.any(np.asarray(inputs["b2"])))
    return with_mask, with_brow


def get_nc(layers=L, with_mask=False, with_brow=False):
    key = (layers, with_mask, with_brow)
    if key not in _NC_CACHE:
        _NC_CACHE[key] = build(layers, with_mask=with_mask, with_brow=with_brow)
    return _NC_CACHE[key]


def run(inputs, layers=L):
    from concourse.bass_utils import run_bass_kernel_spmd
    inputs = {k: np.asarray(v) for k, v in inputs.items()}
    wm, wb = flags_for(inputs)
    nc = get_nc(layers, wm, wb)
    sh = prep_shared(inputs)
    in_maps = [core_inputs(inputs, sh, c) for c in range(NCORES)]
    res = run_bass_kernel_spmd(nc, in_maps, core_ids=list(range(NCORES)))
    outs = [res.results[c]["out"].reshape(B_LOC, S, H) for c in range(NCORES)]
    return np.concatenate(outs, axis=0).astype(np.float32)


def kernel(**inputs):
    return run(inputs)


# revision 22
# speedup vs baseline: 1.2457x; 1.2457x over previous
"""BERT-base forward on 8 Trainium2 NeuronCores.

Strategy: pure data parallelism over the batch (B=16 -> 2 sequences per
core), weights replicated, zero collectives.  Inside each core the
activation stream alternates between token-major layout (for layernorm,
whose reduction runs along the free dimension) and feature-major layout
(for feeding the PE array), with PE-transposes bridging the two.

v9 (vs v8): phase boundaries overlap.  The attention output projection
is emitted inside the attention psum pool right behind the softmax
epilogue; layernorm finalize is split into token-quad halves and the
token->feature transposes are quad-major, so FFN1's nf=0 half starts as
soon as the first 4 token chunks are normalized+transposed while the
other half is still finishing.  Same structure at the FFN2->LN2
boundary so the next layer's V/QK projections overlap the LN2 tail.
One layer-wide PSUM pool: tag "sc" = 2x 4KiB slots (scoresT pairs,
out-proj, FFN2), tag "cx" = 4x 2KiB slots (QK proj, ctx, transposes,
FFN1).

v8: attention is software-pipelined by head pair with fine-grained
interleave: scores(p) units (2 concurrent matmuls + ONE fused exp over
both heads of the pair) are spaced by filler units [ctx(p-1), QK(p+1),
ctxT-transpose(p-2)], keeping the PE dense while the ScalarE exp
conveyor drains.  scores are computed TRANSPOSED
(scoresT[k,q] = K_slice.T @ Q) so exp output is already in ctx-matmul
lhsT layout and no probs transpose is needed; V carries a ones column
per head so the ctx psum col 64 holds the softmax row-sum and
normalization is a per-partition DVE multiply.  Layernorm: batched -
centered square-sum via ScalarE Square(bias=-mean) (table-set filler,
no ACT_TABLE_LOAD), one Sqrt per quad + DVE reciprocal.

Precision: matmul operands are bf16 (PSUM accumulation is fp32), the
residual stream / layernorm statistics / softmax stay fp32.

Layout conventions per core (P=128 partitions):
  tokens NT=1024 (2 seqs x 512), token chunk tc in [0,8)
  features H=768, feature chunk hc in [0,6); FFN I=3072, ic in [0,24)
  token-major  [128 tokens, H]  - residual stream, layernorm, ctx, V
  feature-major [128 features, NT] - matmul lhsT/rhs operands
  matmul computes out = lhsT.T @ rhs (contraction along partitions)
"""

import numpy as np
import ml_dtypes

V, H, L, NH, I, S = 30522, 768, 12, 12, 3072, 512
B_FULL, NCORES, B_LOC = 16, 8, 2
DH = H // NH                      # 64
P = 128
NT = B_LOC * S                    # 1024 tokens per core
TC = NT // P                      # 8 token chunks
HC = H // P                       # 6 feature chunks
IC = I // P                       # 24 ffn chunks
SC = S // P                       # 4 chunks per sequence
EPS = 1e-12
INV_SQRT_DH = 1.0 / 8.0

_BF16 = ml_dtypes.bfloat16


# --------------------------------------------------------------------------
# device kernel builder
# --------------------------------------------------------------------------

def build(layers=L, taps=None, with_mask=False, with_brow=False):
    import concourse.bass as bass
    import concourse.mybir as mybir
    import concourse.tile as tile
    from concourse import bacc
    from contextlib import ExitStack

    dt = mybir.dt
    AF = mybir.ActivationFunctionType
    OP = mybir.AluOpType

    nc = bacc.Bacc("TRN2", target_bir_lowering=False, debug=False,
                   num_devices=NCORES)

    # ---- DRAM inputs (per core) ----
    identm = nc.dram_tensor("identm", [P, P], dt.bfloat16, kind="ExternalInput")
    wrows = nc.dram_tensor("wrows", [NT, H], dt.bfloat16, kind="ExternalInput")
    trows = nc.dram_tensor("trows", [NT, H], dt.bfloat16, kind="ExternalInput")
    pemb = nc.dram_tensor("pemb", [S, H], dt.float32, kind="ExternalInput")
    extm = nc.dram_tensor("extm", [1, B_LOC * S], dt.float32, kind="ExternalInput")
    dWq = nc.dram_tensor("Wq", [L, H, H], dt.bfloat16, kind="ExternalInput")
    dWk = nc.dram_tensor("Wk", [L, H, H], dt.bfloat16, kind="ExternalInput")
    dWv = nc.dram_tensor("Wv", [L, H, H], dt.bfloat16, kind="ExternalInput")
    dWo = nc.dram_tensor("Wo", [L, H, H], dt.bfloat16, kind="ExternalInput")
    dW1 = nc.dram_tensor("W1", [L, H, I], dt.bfloat16, kind="ExternalInput")
    dW2 = nc.dram_tensor("W2", [L, I, H], dt.bfloat16, kind="ExternalInput")
    # per-partition biases: bq is pre-scaled by 1/sqrt(DH) host-side
    dbq = nc.dram_tensor("bq8", [L, H], dt.float32, kind="ExternalInput")
    dbk = nc.dram_tensor("bk", [L, H], dt.float32, kind="ExternalInput")
    db1 = nc.dram_tensor("b1", [L, I], dt.float32, kind="ExternalInput")
    # free-dim biases (added via K=1 rank-1 matmuls): rows [bo + bv@Wo, b2]
    dbrow = nc.dram_tensor("brow", [L, 1, 2 * H], dt.bfloat16, kind="ExternalInput")
    out = nc.dram_tensor("out", [NT, H], dt.float32, kind="ExternalOutput")

    f32, bf16 = dt.float32, dt.bfloat16

    def tap(name, tiles):
        if taps is None:
            return
        sh0 = list(tiles[0].shape)
        d = nc.dram_tensor(f"tap_{name}", [len(tiles)] + sh0,
                           tiles[0].dtype, kind="ExternalOutput")
        for i, t in enumerate(tiles):
            nc.sync.dma_start(d.ap()[i], t[:])
        taps[name] = d

    with tile.TileContext(nc) as tc_, ExitStack() as top:
        tc = tc_

        # ---- constants & persistent activation tiles ----
        pers = top.enter_context(tc.tile_pool(name="pers", bufs=1))
        ident = pers.tile([P, P], bf16, name="ident")
        nc.sync.dma_start(ident[:], identm.ap())
        ones1 = pers.tile([1, P], bf16, name="ones1")
        nc.vector.memset(ones1[:], 1.0)
        eps_t = pers.tile([P, 1], f32, name="eps_t")
        nc.vector.memset(eps_t[:], EPS)

        curA = [pers.tile([P, H], f32, name=f"curA{t}") for t in range(TC)]
        curB = [pers.tile([P, H], f32, name=f"curB{t}") for t in range(TC)]
        xtok = [pers.tile([P, H], bf16, name=f"xtok{t}") for t in range(TC)]
        xT = [pers.tile([P, NT], bf16, name=f"xT{h}") for h in range(HC)]
        # token-major V with a ones column per head: [:, hd, 0:64]=V,
        # [:, hd, 64]=1.0 (set once; V copies never touch col 64)
        vaug = [pers.tile([P, NH, DH + 1], bf16, name=f"vaug{t}")
                for t in range(TC)]
        for t in range(TC):
            nc.vector.memset(vaug[t][:], 1.0)

        small = top.enter_context(tc.tile_pool(name="small", bufs=6))

        # next-layer Wq/Wv prefetch pool: layer l+1's tiles are DMA'd during
        # layer l's FFN phase (before FFN2's W2 burst) so the attention
        # prologue never waits on weight DMA.
        pre_pool = top.enter_context(tc.tile_pool(name="prew", bufs=12))
        pre_w = {}

        def prefetch_qv(l):
            if l >= layers:
                return
            wq = [pre_pool.tile([P, H], bf16, tag="pw", name=f"pwq{l}_{h}")
                  for h in range(HC)]
            wv = [pre_pool.tile([P, H], bf16, tag="pw", name=f"pwv{l}_{h}")
                  for h in range(HC)]
            for h in range(HC):
                nc.sync.dma_start(wq[h][:], dWq.ap()[l, h * P:(h + 1) * P, :])
                nc.sync.dma_start(wv[h][:], dWv.ap()[l, h * P:(h + 1) * P, :])
            pre_w[l] = (wq, wv)

        # per-key-chunk mask columns for the exp bias (token-major [P, TC])
        mask_cols = None
        if with_mask:
            extm_sb = pers.tile([1, B_LOC * S], f32, name="extm_sb")
            nc.sync.dma_start(extm_sb[:], extm.ap())
            mask_cols = pers.tile([P, TC], f32, name="mask_cols")
            ones_bf = pers.tile([1, 1], bf16, name="ones_bf")
            nc.vector.memset(ones_bf[:], 1.0)
            extm_bf = pers.tile([1, B_LOC * S], bf16, name="extm_bf")
            nc.vector.tensor_copy(extm_bf[:], extm_sb[:])
            with ExitStack() as ms:
                mp = ms.enter_context(
                    tc.tile_pool(name="maskp", space="PSUM", bufs=1))
                for t in range(TC):
                    mps = mp.tile([P, 1], f32, tag="m", bufs=2, name="mps")
                    nc.tensor.matmul(
                        mps[:], lhsT=extm_bf[0:1, t * P:(t + 1) * P],
                        rhs=ones_bf[:], start=True, stop=True)
                    nc.scalar.copy(mask_cols[:, t:t + 1], mps[:])

        # ---------------- helpers ----------------
        # Batched layernorm with partial (token-quad) finalize.
        class LNBatch:
            def __init__(self):
                self.uneg = small.tile([P, TC], f32, tag="uneg", bufs=2,
                                       name="uneg")
                self.svar = small.tile([P, TC], f32, tag="svar", bufs=2,
                                       name="svar")
                self.rstd = small.tile([P, TC], f32, tag="rstd", bufs=2,
                                       name="rstd")
                self.items = []

            def add(self, src_ap, res_ap, dst, tcid, last=False, out_f32=None):
                j = len(self.items)
                s1 = small.tile([P, 1], f32, tag="s1")
                nc.vector.scalar_tensor_tensor(
                    out=dst[:], in0=src_ap, scalar=0.0, in1=res_ap,
                    op0=OP.add, op1=OP.add, accum_out=s1[:])
                nc.vector.tensor_scalar(
                    out=self.uneg[:, j:j + 1], in0=s1[:], scalar1=-1.0 / H,
                    scalar2=None, op0=OP.mult)
                junk = small.tile([P, H], f32, tag="junk", bufs=1)
                nc.scalar.activation(junk[:], dst[:], AF.Square,
                                     bias=self.uneg[:, j:j + 1],
                                     accum_out=self.svar[:, j:j + 1])
                self.items.append((dst, tcid, last, out_f32))

            def finish(self, a, b):
                sd = small.tile([P, TC], f32, tag="sd", bufs=2, name="sd")
                nc.scalar.activation(sd[:, a:b], self.svar[:, a:b], AF.Sqrt,
                                     bias=eps_t[:], scale=1.0 / H)
                nc.vector.reciprocal(self.rstd[:, a:b], sd[:, a:b])
                for j in range(a, b):
                    dst, tcid, last, out_f32 = self.items[j]
                    nc.vector.tensor_scalar(
                        out=dst[:], in0=dst[:], scalar1=self.uneg[:, j:j + 1],
                        scalar2=self.rstd[:, j:j + 1],
                        op0=OP.add, op1=OP.mult)
                    if last:
                        nc.sync.dma_start(out_f32, dst[:])
                    else:
                        nc.vector.tensor_copy(xtok[tcid][:], dst[:])

        def trans_quad(pool, tag, bufs, srcs, dsts, tq, keng=0):
            """One token quad (4 chunks) of token-major srcs -> columns
            [tq*512, (tq+1)*512) of all 6 feature-major dsts."""
            for h in range(HC):
                pt = pool.tile([P, 4, P], f32, tag=tag, bufs=bufs, name="pt")
                for j in range(4):
                    t = tq * 4 + j
                    nc.tensor.matmul(pt[:, j, :],
                                     lhsT=srcs[t][:, h * P:(h + 1) * P],
                                     rhs=ident[:], start=True, stop=True)
                dst = dsts[h][:, tq * 4 * P:(tq * 4 + 4) * P]
                src = pt[:].rearrange("p a b -> p (a b)")
                if (h + keng) % 2 == 0:
                    nc.vector.tensor_copy(dst, src)
                else:
                    nc.scalar.copy(dst, src)

        # ---- embedding: gather + add + LN ----
        with ExitStack() as emb_scope:
            ep = emb_scope.enter_context(tc.tile_pool(name="emb", bufs=1))
            epp = emb_scope.enter_context(
                tc.tile_pool(name="embp", space="PSUM", bufs=1))
            wg = ep.tile([P, TC, H], bf16, name="wg")
            tg = ep.tile([P, TC, H], bf16, name="tg")
            nc.sync.dma_start(wg[:], wrows.ap().rearrange("(c p) h -> p c h", p=P))
            nc.sync.dma_start(tg[:], trows.ap().rearrange("(c p) h -> p c h", p=P))
            pos = ep.tile([P, SC, H], f32, name="pos")
            nc.sync.dma_start(pos[:], pemb.ap().rearrange("(c p) h -> p c h", p=P))
            prefetch_qv(0)
            lb = LNBatch()
            for t in range(TC):
                tmp = ep.tile([P, H], f32, tag="etmp", bufs=2, name="etmp")
                nc.vector.tensor_add(tmp[:], tg[:, t], pos[:, t % SC])
                lb.add(wg[:, t], tmp[:], curA[t], t)
            lb.finish(0, TC)
            for tq in range(2):
                trans_quad(epp, "t", 2, xtok, xT, tq)
            tap("emb", curA)
            tap("embxT", xT)

        # ---- transformer layers ----
        for l in range(layers):
            with ExitStack() as ls:
                wp = ls.enter_context(tc.tile_pool(name=f"w{l}", bufs=1))
                # per-partition bias tiles for this layer
                bq_t = wp.tile([P, HC], f32, name=f"bq{l}")
                bk_t = wp.tile([P, HC], f32, name=f"bk{l}")
                b1_t = wp.tile([P, IC], f32, name=f"b1{l}")
                nc.sync.dma_start(bq_t[:], dbq.ap()[l].rearrange("(c p) -> p c", p=P))
                nc.sync.dma_start(bk_t[:], dbk.ap()[l].rearrange("(c p) -> p c", p=P))
                nc.sync.dma_start(b1_t[:], db1.ap()[l].rearrange("(c p) -> p c", p=P))
                brow_t = wp.tile([1, 2 * H], bf16, name=f"brow{l}")
                nc.sync.dma_start(brow_t[:], dbrow.ap()[l])

                cur, nxt = (curA, curB)

                with ExitStack() as attn_scope:
                    psA = attn_scope.enter_context(
                        tc.tile_pool(name=f"psA{l}", space="PSUM", bufs=1))

                    def sc_tile(shape, name):
                        return psA.tile(shape, f32, tag="sc", bufs=2, name=name)

                    def cx_tile(shape, name):
                        return psA.tile(shape, f32, tag="cx", bufs=4, name=name)

                    ap_ = attn_scope.enter_context(
                        tc.tile_pool(name=f"attn{l}", bufs=1))
                    wpool = attn_scope.enter_context(
                        tc.tile_pool(name=f"wqkv{l}", bufs=12))

                    QT = [ap_.tile([P, NT], bf16, name=f"QT{l}_{h}") for h in range(HC)]
                    KT = [ap_.tile([P, NT], bf16, name=f"KT{l}_{h}") for h in range(HC)]
                    ctok = [ap_.tile([P, H], bf16, name=f"ctok{l}_{t}")
                            for t in range(TC)]
                    ctxT = [ap_.tile([P, NT], bf16, name=f"cT{l}_{h}") for h in range(HC)]

                    wqch, wvch = pre_w.pop(l)
                    wkch = [wpool.tile([P, H], bf16, tag="wc",
                                       name=f"wk{l}_{h}") for h in range(HC)]
                    for h in range(HC):
                        nc.sync.dma_start(wkch[h][:], dWk.ap()[l, h * P:(h + 1) * P, :])

                    # ---- emission units for the software pipeline ----
                    def v_unit(t, spec):
                        nf, n0, nn, h0, nh = spec
                        ps = sc_tile([P, S], "psv")
                        for hi in range(HC):
                            nc.tensor.matmul(
                                ps[:, :nn],
                                lhsT=xT[hi][:, t * P:(t + 1) * P],
                                rhs=wvch[hi][:, n0:n0 + nn],
                                start=(hi == 0), stop=(hi == HC - 1))
                        nc.vector.tensor_copy(
                            vaug[t][:, h0:h0 + nh, 0:DH],
                            ps[:, :nn].rearrange("p (h d) -> p h d", h=nh))

                    def qk_unit(p, dW_ch, bt, scale, dstT, nf):
                        ps = cx_tile([P, S], "psqk")
                        for hi in range(HC):
                            nc.tensor.matmul(
                                ps[:],
                                lhsT=dW_ch[hi][:, p * P:(p + 1) * P],
                                rhs=xT[hi][:, nf * S:(nf + 1) * S],
                                start=(hi == 0), stop=(hi == HC - 1))
                        nc.vector.tensor_scalar(
                            out=dstT[p][:, nf * S:(nf + 1) * S],
                            in0=ps[:], scalar1=bt[:, p:p + 1],
                            scalar2=float(scale), op0=OP.add, op1=OP.mult)

                    QK_SPECS = ((wqch, bq_t, INV_SQRT_DH, QT),
                                (wkch, bk_t, 1.0, KT))

                    def qk_units(p, which=(0, 1), nfs=(0, 1)):
                        return [(lambda dc=dc, bt=bt, sc_=sc_, dT=dT,
                                 nf=nf: qk_unit(p, dc, bt, sc_, dT, nf))
                                for dc, bt, sc_, dT in
                                [QK_SPECS[w] for w in which]
                                for nf in nfs]

                    def ctx_unit(ex2, s, side, hd):
                        cx = cx_tile([P, SC, DH + 1], "cx")
                        for qc in range(SC):
                            for kc in range(SC):
                                nc.tensor.matmul(
                                    cx[:, qc, :],
                                    lhsT=ex2[:, kc, side, qc * P:(qc + 1) * P],
                                    rhs=vaug[s * SC + kc][:, hd, :],
                                    start=(kc == 0), stop=(kc == SC - 1))
                        cxs = small.tile([P, SC, DH + 1], f32, tag="cxs",
                                         bufs=4)
                        nc.vector.tensor_copy(cxs[:], cx[:])
                        rinv = small.tile([P, SC], f32, tag="rinv", bufs=4)
                        nc.vector.reciprocal(rinv[:], cxs[:, :, DH])
                        for qc in range(SC):
                            nc.vector.tensor_scalar(
                                out=ctok[s * SC + qc][:, hd * DH:(hd + 1) * DH],
                                in0=cxs[:, qc, 0:DH],
                                scalar1=rinv[:, qc:qc + 1],
                                scalar2=None, op0=OP.mult)

                    def trans_unit(h):
                        # ctxT[h] only needs head pair h of ctok
                        for tq in range(TC // 4):
                            pt = cx_tile([P, 4, P], "ptc")
                            for j in range(4):
                                t = tq * 4 + j
                                nc.tensor.matmul(
                                    pt[:, j, :],
                                    lhsT=ctok[t][:, h * P:(h + 1) * P],
                                    rhs=ident[:], start=True, stop=True)
                            nc.vector.tensor_copy(
                                ctxT[h][:, tq * 4 * P:(tq * 4 + 4) * P],
                                pt[:].rearrange("p a b -> p (a b)"))

                    # ---- prologue, ordered by token quad and DMA arrival:
                    # Q/V use prefetched weights; K's DMA finishes during V
                    for u in qk_units(0, which=(0,), nfs=(0,)):
                        u()
                    for t in range(SC):
                        v_unit(t, (0, 0, S, 0, 8))
                        v_unit(t, (1, S, H - S, 8, 4))
                    for u in qk_units(0, which=(0,), nfs=(1,)):
                        u()
                    for t in range(SC, TC):
                        v_unit(t, (0, 0, S, 0, 8))
                        v_unit(t, (1, S, H - S, 8, 4))
                    for u in qk_units(0, which=(1,)):
                        u()

                    ex_pend = None   # (ex2 tiles, pair) awaiting ctx
                    for p in range(HC):
                        hA, hB = 2 * p, 2 * p + 1
                        fillers = []
                        if p == 0:
                            fillers += qk_units(1)
                        else:
                            pe, pp = ex_pend
                            for s in range(B_LOC):
                                for side, hd in ((0, 2 * pp), (1, 2 * pp + 1)):
                                    fillers.append(
                                        lambda e=pe[s], s=s, sd=side,
                                        hd=hd: ctx_unit(e, s, sd, hd))
                            if p + 1 < HC:
                                fillers += qk_units(p + 1)
                            if p - 2 >= 0:
                                fillers.append(lambda h=p - 2: trans_unit(h))

                        # score units (s, kc): 2 concurrent matmuls into a
                        # 2-bank psum slot, ONE fused exp over both heads
                        ex2s = [None, None]
                        nslots = 2 * SC
                        fi = 0
                        for s in range(B_LOC):
                            ex2s[s] = ap_.tile([P, SC, 2, S], bf16,
                                               tag="ex", bufs=3, name="ex2")
                            for kc in range(SC):
                                ps2 = sc_tile([P, 2, S], "ps2")
                                for side, hd in ((0, hA), (1, hB)):
                                    po = (hd % 2) * DH
                                    nc.tensor.matmul(
                                        ps2[:, side, :],
                                        lhsT=KT[p][po:po + DH,
                                                   s * S + kc * P:
                                                   s * S + (kc + 1) * P],
                                        rhs=QT[p][po:po + DH,
                                                  s * S:(s + 1) * S],
                                        start=True, stop=True,
                                        tile_position=(po, 0))
                                bias = (mask_cols[:, s * SC + kc:
                                                  s * SC + kc + 1]
                                        if with_mask else 0.0)
                                nc.scalar.activation(
                                    ex2s[s][:, kc, :, :].rearrange(
                                        "p a b -> p (a b)"),
                                    ps2[:].rearrange("p a b -> p (a b)"),
                                    AF.Exp, bias=bias)
                                # pop interleaved fillers
                                slot = s * SC + kc
                                want = ((slot + 1) * len(fillers)) // nslots
                                while fi < want:
                                    fillers[fi]()
                                    fi += 1
                        while fi < len(fillers):
                            fillers[fi]()
                            fi += 1
                        ex_pend = (ex2s, p)

                    # epilogue: last ctx + ctxT transposes
                    pe, pp = ex_pend
                    for s in range(B_LOC):
                        for side, hd in ((0, 2 * pp), (1, 2 * pp + 1)):
                            ctx_unit(pe[s], s, side, hd)
                    trans_unit(HC - 2)
                    trans_unit(HC - 1)

                    if l == 0:
                        tap("QT", QT)
                        tap("KT", KT)
                        tap("V", vaug)
                        tap("ctok", ctok)
                        tap("ctxT", ctxT)

                    # --- output projection + LN1, quad-split finalize ---
                    woch = [wpool.tile([P, H], bf16, tag="wc",
                                       name=f"wo{l}_{h}") for h in range(HC)]
                    for h in range(HC):
                        nc.sync.dma_start(woch[h][:], dWo.ap()[l, h * P:(h + 1) * P, :])
                    lb = LNBatch()

                    def outproj_unit(t):
                        po_ = sc_tile([P, H], "po")
                        for nf, n0, nn in ((0, 0, S), (1, S, H - S)):
                            for hi in range(HC):
                                nc.tensor.matmul(
                                    po_[:, n0:n0 + nn],
                                    lhsT=ctxT[hi][:, t * P:(t + 1) * P],
                                    rhs=woch[hi][:, n0:n0 + nn],
                                    start=(hi == 0),
                                    stop=(hi == HC - 1 and not with_brow))
                            if with_brow:
                                nc.tensor.matmul(po_[:, n0:n0 + nn], lhsT=ones1[:],
                                                 rhs=brow_t[0:1, n0:n0 + nn],
                                                 start=False, stop=True)
                        lb.add(po_[:], cur[t][:], nxt[t], t)

                    for t in range(6):
                        outproj_unit(t)
                    lb.finish(0, 4)
                    outproj_unit(6)
                    outproj_unit(7)
                    lb.finish(4, 8)
                    trans_quad(psA, "cx", 4, xtok, xT, 0, keng=0)
                    if l == 0:
                        tap("ln1", nxt)

                # --- FFN ---
                with ExitStack() as ffn_scope:
                    psF = ffn_scope.enter_context(
                        tc.tile_pool(name=f"psF{l}", space="PSUM", bufs=1))
                    fp_ = ffn_scope.enter_context(
                        tc.tile_pool(name=f"ffn{l}", bufs=1))
                    w1pool = ffn_scope.enter_context(
                        tc.tile_pool(name=f"w1p{l}", bufs=6))
                    w2pool = ffn_scope.enter_context(
                        tc.tile_pool(name=f"w2p{l}", bufs=6))

                    def f_tile(shape, name):
                        return psF.tile(shape, f32, tag="f", bufs=4, name=name)

                    gT = [fp_.tile([P, NT], bf16, name=f"gT{l}_{i}") for i in range(IC)]
                    w1ch = [w1pool.tile([P, I], bf16, tag="w1c",
                                        name=f"w1{l}_{h}") for h in range(HC)]
                    for h in range(HC):
                        nc.sync.dma_start(w1ch[h][:], dW1.ap()[l, h * P:(h + 1) * P, :])
                    prefetch_qv(l + 1)

                    def ffn1_half(nf):
                        for i_ in range(IC):
                            ps = f_tile([P, S], "psf")
                            for hi in range(HC):
                                nc.tensor.matmul(
                                    ps[:],
                                    lhsT=w1ch[hi][:, i_ * P:(i_ + 1) * P],
                                    rhs=xT[hi][:, nf * S:(nf + 1) * S],
                                    start=(hi == 0), stop=(hi == HC - 1))
                            nc.scalar.activation(
                                gT[i_][:, nf * S:(nf + 1) * S], ps[:], AF.Gelu,
                                bias=b1_t[:, i_:i_ + 1], scale=1.0)

                    ffn1_half(0)
                    # second token quad of LN1 output -> xT while FFN1 nf=0 runs
                    trans_quad(psF, "f", 4, xtok, xT, 1, keng=0)
                    ffn1_half(1)

                    if l == 0:
                        tap("gT", gT)

                    # ffn2: 4 token chunks per W2 sweep (all 8 psum banks),
                    # LN2 finalize and transposes per token quad
                    lb = LNBatch()
                    last = (l == layers - 1)
                    for sw in range(2):
                        w2ch = [w2pool.tile([P, H], bf16, tag="w2c",
                                            name=f"w2_{l}_{sw}_{i}") for i in range(IC)]
                        pf = [f_tile([P, H], f"pf{t}") for t in range(4)]
                        for i_ in range(IC):
                            nc.sync.dma_start(w2ch[i_][:], dW2.ap()[l, i_ * P:(i_ + 1) * P, :])
                        for i_ in range(IC):
                            for t in range(4):
                                tt = sw * 4 + t
                                for nf, n0, nn in ((0, 0, S), (1, S, H - S)):
                                    nc.tensor.matmul(
                                        pf[t][:, n0:n0 + nn],
                                        lhsT=gT[i_][:, tt * P:(tt + 1) * P],
                                        rhs=w2ch[i_][:, n0:n0 + nn],
                                        start=(i_ == 0),
                                        stop=(i_ == IC - 1 and not with_brow))
                        for t in range(4):
                            tt = sw * 4 + t
                            if with_brow:
                                for nf, n0, nn in ((0, 0, S), (1, S, H - S)):
                                    nc.tensor.matmul(pf[t][:, n0:n0 + nn],
                                                     lhsT=ones1[:],
                                                     rhs=brow_t[0:1, H + n0:H + n0 + nn],
                                                     start=False, stop=True)
                            lb.add(pf[t][:], nxt[tt][:], cur[tt], tt, last=last,
                                   out_f32=out.ap()[tt * P:(tt + 1) * P, :])
                        lb.finish(sw * 4, sw * 4 + 4)
                        if not last:
                            trans_quad(psF, "f", 4, xtok, xT, sw, keng=1)

    nc.compile()
    return nc


# --------------------------------------------------------------------------
# host side
# --------------------------------------------------------------------------

def prep_shared(inputs):
    sh = {}
    sh["identm"] = np.eye(P, dtype=_BF16)
    sh["wemb_bf"] = inputs["word_emb"].astype(_BF16)
    sh["temb_bf"] = inputs["type_emb"].astype(_BF16)
    sh["pemb"] = inputs["pos_emb"].astype(np.float32)
    for k in ("Wq", "Wk", "Wv", "Wo", "W1", "W2"):
        sh[k] = inputs[k].astype(_BF16)
    sh["bq8"] = (inputs["bq"] * INV_SQRT_DH).astype(np.float32)
    sh["bk"] = inputs["bk"].astype(np.float32)
    sh["b1"] = inputs["b1"].astype(np.float32)
    # bv is folded into the attn-out row bias: (ctx+bv)@Wo + bo
    #   = ctx@Wo + (bv@Wo + bo)
    bo_eff = np.einsum("lh,lho->lo", inputs["bv"].astype(np.float64),
                       inputs["Wo"].astype(np.float64)) + inputs["bo"]
    sh["brow"] = np.concatenate(
        [bo_eff.astype(np.float32), inputs["b2"]], axis=1)[:, None, :].astype(_BF16)
    return sh


def core_inputs(inputs, sh, c):
    ids = np.asarray(inputs["input_ids"]).astype(np.int64)
    tts = np.asarray(inputs["token_type_ids"]).astype(np.int64)
    am = np.asarray(inputs["attention_mask"]).astype(np.float32)
    b0 = c * B_LOC
    m = {k: v for k, v in sh.items() if k not in ("wemb_bf", "temb_bf")}
    m["wrows"] = np.ascontiguousarray(sh["wemb_bf"][ids[b0:b0 + B_LOC].reshape(-1)])
    m["trows"] = np.ascontiguousarray(sh["temb_bf"][tts[b0:b0 + B_LOC].reshape(-1)])
    m["extm"] = ((1.0 - am[b0:b0 + B_LOC]) * -10000.0).reshape(1, -1).astype(np.float32)
    return m


_NC_CACHE = {}


def flags_for(inputs):
    with_mask = not np.all(np.asarray(inputs["attention_mask"]) == 1.0)
    with_brow = bool(np.any(np.asarray(inputs["bo"])) or
                     np.any(np.asarray(inputs["bv"])) or
                     np.any(np.asarray(inputs["b2"])))
    return with_mask, with_brow


def get_nc(layers=L, with_mask=False, with_brow=False):
    key = (layers, with_mask, with_brow)
    if key not in _NC_CACHE:
        _NC_CACHE[key] = build(layers, with_mask=with_mask, with_brow=with_brow)
    return _NC_CACHE[key]


def run(inputs, layers=L):
    from concourse.bass_utils import run_bass_kernel_spmd
    inputs = {k: np.asarray(v) for k, v in inputs.items()}
    wm, wb = flags_for(inputs)
    nc = get_nc(layers, wm, wb)
    sh = prep_shared(inputs)
    in_maps = [core_inputs(inputs, sh, c) for c in range(NCORES)]
    res = run_bass_kernel_spmd(nc, in_maps, core_ids=list(range(NCORES)))
    outs = [res.results[c]["out"].reshape(B_LOC, S, H) for c in range(NCORES)]
    return np.concatenate(outs, axis=0).astype(np.float32)


def kernel(**inputs):
    return run(inputs)
